# revision 32
# baseline (speedup 1.0000x reference)
"""Trainium2 Bass kernel for nn_CrossPatchContextModule.

Math (per batch b):
    hi = x @ W1[:D];  hj = x @ W1[D:]
    scores[i,j] = W2 . relu(hi[i] + hj[j] + b1) + b2     (diag forced to 0)
    w = softmax(scores, axis=j)
    out = x + LN(w @ x @ Wp + bp) * gamma + beta

Sharding: data-parallel over batch. B=8 batches -> 8 NeuronCores, one
batch per core, all parameters replicated. No collectives.

Per-core algorithm (N=D=256, P=128 partitions):
  * Fold a = |W2| into W1 on the host (W1' = W1 * a), so the pairwise relu
    tile R[e, (i,j)] = relu(a_e*(hi+hj+b1)) = a_e*relu(hi+hj+b1) is produced
    in ONE tensor_scalar (DVE) / activation-Relu (ACT) op per (i, e-chunk):
    in0 = hjbW (j on free axis), per-partition scalar = hiW[:, i].
  * scores[i, :] = sum_e sign(W2)_e * R[e, :]. Contract on the PE with a
    shifted-window one-hot weight: sb32 is [128, 64] with column 32 equal
    to sign(W2) for that e-chunk and zeros elsewhere; sb32[:, 32-m:64-m]
    is a [128,32] matrix whose only nonzero column is m = p%32, so with the
    output sliced to partitions [32g, 32g+32) (g = p//32, tile_position
    col-group g) the matvec lands on PSUM partition p while accumulating +0
    onto the other rows of its group. Rows are emitted round-robin over the
    4 column groups so consecutive matmuls execute concurrently in disjoint
    32-column strips of the PE array. All 256 matmuls form one accumulation
    group into a single PSUM bank => scores end up dense [i(part), j(free)]
    with rows (p, p+128) packed as the two 256-wide halves of the free axis.
  * softmax: fused (scores+b2)*mask via scalar_tensor_tensor straight
    from PSUM, ACT Exp (fp16 out) with accum_out giving row sums for
    free, reciprocal + per-partition scale; processed per i-half so the
    two halves pipeline across engines.
  * ctx^T = x(chunks as lhsT) @ w^T (w transposed 128x128 via PE),
    proj = ctx^T(lhsT) @ Wp -> [i(part), e(free)], + bp, LayerNorm via
    bn_stats/bn_aggr, rstd = Exp(-0.5*Ln(var+eps)), residual add.

R tiles, the hi/hj matmul operands, and the softmax/ctx/proj weights are
fp16 (PE runs fp32 matmuls at 1/4 speed; fp16 streams at 1 col/cycle and
DVE tensor_scalar gets the 16-bit perf modes). Scores accumulate in fp32
PSUM; LayerNorm statistics and the residual stay fp32.
"""

import numpy as np
from contextlib import ExitStack

import concourse.bass as bass
import concourse.bacc as bacc
import concourse.tile as tile
from concourse import mybir
from concourse.bass_utils import run_bass_kernel_spmd

B, N, D = 8, 256, 256
P = 128
LN_EPS = 1e-5
F32 = mybir.dt.float32
F16 = mybir.dt.float16
AF = mybir.ActivationFunctionType
OP = mybir.AluOpType

# Per-row engine assignment for the pairwise relu tiles, balancing the
# three elementwise-capable engines (HW rates per [128,256] op: DVE
# ~127ns at the 16-bit perf mode, ACT ~400ns, Pool ~500ns): out of every
# 32 rows, 21 go to DVE, 7 to ACT, 4 to Pool.
def _row_engine(nc, p):
    q, k = p % 32, p // 32
    if q >= 2 and k == q % 4 and (q % 8) != 7:
        return nc.scalar
    if q >= 8 and q % 2 == 0 and k == (q + 2) % 4:
        return nc.gpsimd
    return nc.vector


def _single_act_table(arch):
    """All activation funcs this kernel uses (Relu/Identity/Copy/Exp/Ln)
    live in set 6 (natural_log_exp_and_others). The stock greedy table
    placer picks sets 0/5 and ping-pongs 5 table loads (~2.7us each on
    HW); masking every other set forces one load of set 6. Canonical set
    indices are preserved (walrus maps id -> act.json by position)."""
    import concourse.hw_specs as hw_specs

    tabs = hw_specs.get_activation_tables(arch)
    keep = "natural_log_exp_and_others"
    need = {AF.Relu, AF.Identity, AF.Copy, AF.Exp, AF.Ln}
    if keep not in tabs or not need.issubset(tabs[keep]):
        return tabs  # fall back to the stock placement
    return {name: (funcs if name == keep else set()) for name, funcs in tabs.items()}


def _build_program(b2_val: float, use_gamma: bool, use_beta: bool):
    nc = bacc.Bacc("TRN2", target_bir_lowering=False, debug=False)

    xb_d = nc.dram_tensor("xb", [N, D], F32, kind="ExternalInput")
    xt16_d = nc.dram_tensor("xt16", [D, N], F16, kind="ExternalInput")
    x16_d = nc.dram_tensor("x16", [N, D], F16, kind="ExternalInput")
    w1a_d = nc.dram_tensor("w1a", [D, D], F16, kind="ExternalInput")
    w1b_d = nc.dram_tensor("w1b", [D, D], F16, kind="ExternalInput")
    ab1_d = nc.dram_tensor("ab1c", [P, 2], F32, kind="ExternalInput")
    sb0_d = nc.dram_tensor("sb0", [P, 64], F16, kind="ExternalInput")
    sb1_d = nc.dram_tensor("sb1", [P, 64], F16, kind="ExternalInput")
    mask_d = nc.dram_tensor("maskd", [P, N * 2], F32, kind="ExternalInput")
    ident16_d = nc.dram_tensor("ident16", [P, P], F16, kind="ExternalInput")
    wp_d = nc.dram_tensor("wp", [D, D], F16, kind="ExternalInput")
    bpr_d = nc.dram_tensor("bpr", [P, D], F32, kind="ExternalInput")
    xpb_d = (
        nc.dram_tensor("xpb", [N, D], F32, kind="ExternalInput")
        if use_beta
        else None
    )
    gam_d = (
        nc.dram_tensor("gamr", [P, D], F32, kind="ExternalInput")
        if use_gamma
        else None
    )
    out_d = nc.dram_tensor("out", [N, D], F32, kind="ExternalOutput")

    with tile.TileContext(nc) as tc, ExitStack() as ctx:
        const = ctx.enter_context(tc.tile_pool(name="const", bufs=1))
        rpool = ctx.enter_context(tc.tile_pool(name="rtiles", bufs=16))
        rslow = ctx.enter_context(tc.tile_pool(name="rslow", bufs=16))
        ppre = ctx.enter_context(tc.tile_pool(name="ppre", bufs=2, space="PSUM"))
        pscore = ctx.enter_context(
            tc.tile_pool(name="pscore", bufs=1, space="PSUM")
        )

        # per-partition scalar constants for activation bias operands
        zero1 = const.tile([P, 1], F32)
        nc.vector.memset(zero1, 0.0)
        eps1 = const.tile([P, 1], F32)
        nc.vector.memset(eps1, LN_EPS)
        b2v = const.tile([P, 1], F32)
        nc.vector.memset(b2v, b2_val)

        # ------- input DMAs needed before the main loop -------------------
        # x^T and the fp16 copy of x are host-side layout transforms of the
        # per-core shard (no FLOPs) - DMA them directly.
        xT = [const.tile([P, N], F16, tag=f"xT{c}", name=f"xT{c}") for c in range(2)]
        for c in range(2):
            nc.sync.dma_start(xT[c], xt16_d[c * P : (c + 1) * P, :])
        w1a = [const.tile([P, D], F16, tag=f"w1a{c}", name=f"w1a{c}") for c in range(2)]
        w1b = [const.tile([P, D], F16, tag=f"w1b{c}", name=f"w1b{c}") for c in range(2)]
        for c in range(2):
            nc.scalar.dma_start(w1a[c], w1a_d[c * P : (c + 1) * P, :])
            nc.gpsimd.dma_start(w1b[c], w1b_d[c * P : (c + 1) * P, :])
        ab1c = const.tile([P, 2], F32)
        nc.sync.dma_start(ab1c, ab1_d[:])
        sb = [const.tile([P, 64], F16, tag=f"sb{c}", name=f"sb{c}") for c in range(2)]
        nc.sync.dma_start(sb[0], sb0_d[:])
        nc.sync.dma_start(sb[1], sb1_d[:])
        # needed only from the softmax stage on; emitted early but behind
        # the critical xT/W1 loads on the queues
        x = [const.tile([P, D], F32, tag=f"x{c}", name=f"x{c}") for c in range(2)]
        nc.sync.dma_start(x[0], xb_d[0:P, :])
        nc.gpsimd.dma_start(x[1], xb_d[P : 2 * P, :])
        x16 = [const.tile([P, D], F16, tag=f"x16{c}", name=f"x16{c}") for c in range(2)]
        for c in range(2):
            nc.gpsimd.dma_start(x16[c], x16_d[c * P : (c + 1) * P, :])
        ident16 = const.tile([P, P], F16)
        nc.sync.dma_start(ident16, ident16_d[:])
        maskd = const.tile([P, 2 * N], F32)
        nc.sync.dma_start(maskd, mask_d[:])
        wp16 = [const.tile([P, D], F16, tag=f"wp{c}", name=f"wp{c}") for c in range(2)]
        for c in range(2):
            nc.sync.dma_start(wp16[c], wp_d[c * P : (c + 1) * P, :])
        bpr = const.tile([P, D], F32)
        nc.sync.dma_start(bpr, bpr_d[:])
        if use_beta:
            xpb = [
                const.tile([P, D], F32, tag=f"xpb{c}", name=f"xpb{c}")
                for c in range(2)
            ]
            for c in range(2):
                nc.sync.dma_start(xpb[c], xpb_d[c * P : (c + 1) * P, :])
        if use_gamma:
            gam = const.tile([P, D], F32)
            nc.sync.dma_start(gam, gam_d[:])

        # ---------------- hiW (fp32 scalars), hjbW (fp16 stream) ----------
        # hiW[e,i] = sum_d (W1a*a)[d,e] x[i,d] ; hjbW[e,j] = ... + a*b1
        hiW = [const.tile([P, N], F32, tag=f"hiW{c}", name=f"hiW{c}") for c in range(2)]
        hjbW = [const.tile([P, N], F16, tag=f"hjbW{c}", name=f"hjbW{c}") for c in range(2)]
        for ec in range(2):
            ph = ppre.tile([P, N], F32, tag="mm")
            for dc in range(2):
                nc.tensor.matmul(
                    ph,
                    w1a[dc][:, ec * P : (ec + 1) * P],
                    xT[dc],
                    start=(dc == 0),
                    stop=(dc == 1),
                )
            nc.vector.tensor_copy(hiW[ec], ph)
        for ec in range(2):
            ph = ppre.tile([P, N], F32, tag="mm")
            for dc in range(2):
                nc.tensor.matmul(
                    ph,
                    w1b[dc][:, ec * P : (ec + 1) * P],
                    xT[dc],
                    start=(dc == 0),
                    stop=(dc == 1),
                )
            # + a*b1 (per-partition bias) while converting to fp16
            nc.scalar.activation(
                hjbW[ec], ph, AF.Identity, bias=ab1c[:, ec : ec + 1]
            )

        # ---------------- pairwise scores --------------------------------
        # psum_s[p, h*256+j] = scores[i=p+128h, j]
        # Rb layout: [c0h0 | c0h1 | c1h0 | c1h1], each 256 wide.
        def emit_row(pool, p):
            rb = pool.tile([P, 4 * N], F16, tag="Rb", name=f"Rb{p}")
            eng = _row_engine(nc, p)
            for c in range(2):
                for h in range(2):
                    i = p + P * h
                    dst = rb[:, (c * 2 + h) * N : (c * 2 + h + 1) * N]
                    if eng is nc.scalar:
                        nc.scalar.activation(
                            dst, hjbW[c], AF.Relu, bias=hiW[c][:, i : i + 1]
                        )
                    else:
                        eng.tensor_scalar(
                            out=dst,
                            in0=hjbW[c],
                            scalar1=hiW[c][:, i : i + 1],
                            scalar2=0.0,
                            op0=OP.add,
                            op1=OP.max,
                        )
            return rb

        # Pool rows are ~4x slower per op than DVE; emit them all up front
        # (their own 16-slot pool) so the Q7 cores grind through them in the
        # background while the PE accumulation chain advances quad by quad.
        pre_rbs = {}
        for p in range(P):
            if _row_engine(nc, p) is nc.gpsimd:
                pre_rbs[p] = emit_row(rslow, p)

        psum_s = pscore.tile([P, 2 * N], F32)
        for q in range(32):
            rows = [q, q + 32, q + 64, q + 96]
            rbs = []
            for k, p in enumerate(rows):
                rbs.append(pre_rbs.pop(p, None) or emit_row(rpool, p))
            for c in range(2):
                for k, p in enumerate(rows):
                    m = p & 31
                    nc.tensor.matmul(
                        psum_s[32 * k : 32 * (k + 1), :],
                        sb[c][:, 32 - m : 64 - m],
                        rbs[k][:, c * 2 * N : (c * 2 + 2) * N],
                        start=(q == 0 and c == 0),
                        stop=(q == 31 and c == 1),
                        tile_position=(0, 32 * k),
                        skip_group_check=True,
                    )

        if not use_beta:
            xpb = x

        # -------- softmax / ctx / proj / LN, pipelined per i-half ---------
        # half hh covers i in [hh*128, hh*128+128) = free columns
        # [hh*256, hh*256+256) of psum_s.
        sm2 = const.tile([P, 2 * N], F32)
        ew = const.tile([P, 2 * N], F16)
        S = const.tile([P, 2], F32)
        recip = const.tile([P, 2], F32)
        wgt = const.tile([P, 2 * N], F16)
        wT = [const.tile([P, N], F16, tag=f"wT{c}", name=f"wT{c}") for c in range(2)]
        ctxT = [const.tile([P, N], F16, tag=f"ctxT{c}", name=f"ctxT{c}") for c in range(2)]
        for hh in range(2):
            hs = slice(hh * N, (hh + 1) * N)
            # sm2 = (scores + b2) * diag_mask, fused, straight from PSUM
            nc.vector.scalar_tensor_tensor(
                out=sm2[:, hs], in0=psum_s[:, hs], scalar=b2_val,
                in1=maskd[:, hs], op0=OP.add, op1=OP.mult,
            )
            nc.scalar.activation(
                ew[:, hs], sm2[:, hs], AF.Exp,
                bias=zero1[:, 0:1], accum_out=S[:, hh : hh + 1],
            )
            nc.vector.reciprocal(recip[:, hh : hh + 1], S[:, hh : hh + 1])
            nc.vector.tensor_scalar(
                out=wgt[:, hs], in0=ew[:, hs],
                scalar1=recip[:, hh : hh + 1], scalar2=None, op0=OP.mult,
            )
            # transpose this half's 128x256 block of w
            for cj in range(2):
                tp16 = ppre.tile([P, P], F16, tag="tp")
                nc.tensor.transpose(
                    tp16, wgt[:, hh * N + cj * P : hh * N + (cj + 1) * P], ident16
                )
                nc.scalar.copy(wT[cj][:, hh * P : (hh + 1) * P], tp16)
            # ctx^T[d, i-half] = sum_j x[j,d] w[i,j]
            for dc in range(2):
                pc = ppre.tile([P, P], F32, tag="mm")
                for jc in range(2):
                    nc.tensor.matmul(
                        pc,
                        x16[jc][:, dc * P : (dc + 1) * P],
                        wT[jc][:, hh * P : (hh + 1) * P],
                        start=(jc == 0),
                        stop=(jc == 1),
                    )
                nc.scalar.copy(ctxT[dc][:, hh * P : (hh + 1) * P], pc)
            # proj[i-half, e] then LayerNorm + residual
            pp = ppre.tile([P, N], F32, tag="mm")
            for dc in range(2):
                nc.tensor.matmul(
                    pp,
                    ctxT[dc][:, hh * P : (hh + 1) * P],
                    wp16[dc],
                    start=(dc == 0),
                    stop=(dc == 1),
                )
            pb = const.tile([P, D], F32, tag=f"pb{hh}", name=f"pb{hh}")
            nc.vector.tensor_tensor(out=pb, in0=pp, in1=bpr, op=OP.add)
            st = const.tile([P, 6], F32, tag=f"st{hh}", name=f"st{hh}")
            nc.vector.bn_stats(st, pb)
            mv = const.tile([P, 2], F32, tag=f"mv{hh}", name=f"mv{hh}")
            nc.vector.bn_aggr(mv, st)
            lnv = const.tile([P, 1], F32, tag=f"lnv{hh}", name=f"lnv{hh}")
            nc.scalar.activation(lnv, mv[:, 1:2], AF.Ln, bias=eps1[:, 0:1])
            rstd = const.tile([P, 1], F32, tag=f"rstd{hh}", name=f"rstd{hh}")
            nc.scalar.activation(rstd, lnv, AF.Exp, bias=zero1[:, 0:1], scale=-0.5)
            nmr = const.tile([P, 1], F32, tag=f"nmr{hh}", name=f"nmr{hh}")
            nc.vector.tensor_scalar(
                out=nmr,
                in0=mv[:, 0:1],
                scalar1=rstd[:, 0:1],
                scalar2=-1.0,
                op0=OP.mult,
                op1=OP.mult,
            )
            tt = const.tile([P, D], F32, tag=f"tt{hh}", name=f"tt{hh}")
            nc.vector.tensor_scalar(
                out=tt,
                in0=pb,
                scalar1=rstd[:, 0:1],
                scalar2=nmr[:, 0:1],
                op0=OP.mult,
                op1=OP.add,
            )
            if use_gamma:
                tg = const.tile([P, D], F32, tag=f"tg{hh}", name=f"tg{hh}")
                nc.vector.tensor_tensor(out=tg, in0=tt, in1=gam, op=OP.mult)
                tt = tg
            ot = const.tile([P, D], F32, tag=f"ot{hh}", name=f"ot{hh}")
            nc.vector.tensor_tensor(out=ot, in0=tt, in1=xpb[hh], op=OP.add)
            nc.sync.dma_start(out_d[hh * P : (hh + 1) * P, :], ot)

    import concourse.bacc as _bacc_mod

    orig = _bacc_mod.get_activation_tables
    _bacc_mod.get_activation_tables = _single_act_table
    try:
        nc.compile()
    finally:
        _bacc_mod.get_activation_tables = orig
    return nc


_cache = {}


def _get_program(b2_val: float, use_gamma: bool, use_beta: bool):
    key = (b2_val, use_gamma, use_beta)
    if key not in _cache:
        _cache[key] = _build_program(b2_val, use_gamma, use_beta)
    return _cache[key]


def _host_inputs(inputs):
    x = np.ascontiguousarray(np.asarray(inputs["patch_features"], np.float32))
    W1 = np.asarray(inputs["W1"], np.float32)
    b1 = np.asarray(inputs["b1"], np.float32)
    W2 = np.asarray(inputs["W2"], np.float32).reshape(-1)
    b2 = float(np.asarray(inputs["b2"], np.float32).reshape(-1)[0])
    Wp = np.ascontiguousarray(np.asarray(inputs["Wp"], np.float32))
    bp = np.asarray(inputs["bp"], np.float32)
    gam = np.asarray(inputs["ln_gamma"], np.float32)
    bet = np.asarray(inputs["ln_beta"], np.float32)

    a = np.abs(W2)
    sig = np.where(W2 >= 0.0, 1.0, -1.0).astype(np.float32)
    w1a = np.ascontiguousarray((W1[:D] * a[None, :]).astype(np.float16))
    w1b = np.ascontiguousarray((W1[D:] * a[None, :]).astype(np.float16))
    ab1c = np.ascontiguousarray((a * b1).reshape(2, P).T)  # [P, 2]
    sbs = []
    for c in range(2):
        m = np.zeros((P, 64), np.float16)
        m[:, 32] = sig[c * P : (c + 1) * P].astype(np.float16)
        sbs.append(m)
    mask = np.ones((P, 2 * N), np.float32)
    for p in range(P):
        mask[p, p] = 0.0
        mask[p, N + P + p] = 0.0
    ident16 = np.eye(P, dtype=np.float16)
    bpr = np.ascontiguousarray(np.broadcast_to(bp[None, :], (P, D)))
    use_gamma = not np.all(gam == 1.0)
    use_beta = not np.all(bet == 0.0)
    gamr = np.ascontiguousarray(np.broadcast_to(gam[None, :], (P, D)))

    common = {
        "w1a": w1a,
        "w1b": w1b,
        "ab1c": ab1c,
        "sb0": sbs[0],
        "sb1": sbs[1],
        "maskd": mask,
        "ident16": ident16,
        "wp": Wp.astype(np.float16),
        "bpr": bpr,
    }
    if use_gamma:
        common["gamr"] = gamr
    in_maps = []
    for b in range(B):
        m = dict(common)
        m["xb"] = np.ascontiguousarray(x[b])
        m["xt16"] = np.ascontiguousarray(x[b].T.astype(np.float16))
        m["x16"] = np.ascontiguousarray(x[b].astype(np.float16))
        if use_beta:
            m["xpb"] = np.ascontiguousarray(x[b] + bet[None, :])
        in_maps.append(m)
    return in_maps, b2, use_gamma, use_beta


def _run(inputs, trace=False, tmpdir=None):
    in_maps, b2, use_gamma, use_beta = _host_inputs(inputs)
    nc = _get_program(b2, use_gamma, use_beta)
    res = run_bass_kernel_spmd(
        nc, in_maps, list(range(B)), trace=trace, tmpdir=tmpdir
    )
    out = np.stack([res.results[b]["out"] for b in range(B)]).astype(np.float32)
    return out, res


def kernel(**inputs) -> np.ndarray:
    out, _ = _run(inputs)
    return out


def predicted_time_ns():
    """Cost-model timeline estimate of one core's NEFF execution (ns)."""
    from concourse.timeline_sim import TimelineSim

    assert _cache, "run the kernel first"
    nc = next(iter(_cache.values()))
    tl = TimelineSim(nc, trace=False)
    return int(tl.simulate())


# revision 41
# speedup vs baseline: 1.0042x; 1.0042x over previous
"""Trainium2 Bass kernel for nn_CrossPatchContextModule.

Math (per batch b):
    hi = x @ W1[:D];  hj = x @ W1[D:]
    scores[i,j] = W2 . relu(hi[i] + hj[j] + b1) + b2     (diag forced to 0)
    w = softmax(scores, axis=j)
    out = x + LN(w @ x @ Wp + bp) * gamma + beta

Sharding: data-parallel over batch. B=8 batches -> 8 NeuronCores, one
batch per core, all parameters replicated. No collectives.

Per-core algorithm (N=D=256, P=128 partitions):
  * Fold a = |W2| into W1 on the host (W1' = W1 * a), so the pairwise relu
    tile R[e, (i,j)] = relu(a_e*(hi+hj+b1)) = a_e*relu(hi+hj+b1) is produced
    in ONE tensor_scalar (DVE) / activation-Relu (ACT) op per (i, e-chunk):
    in0 = hjbW (j on free axis), per-partition scalar = hiW[:, i].
  * scores[i, :] = sum_e sign(W2)_e * R[e, :]. Contract on the PE with a
    shifted-window one-hot weight: sb32 is [128, 64] with column 32 equal
    to sign(W2) for that e-chunk and zeros elsewhere; sb32[:, 32-m:64-m]
    is a [128,32] matrix whose only nonzero column is m = p%32, so with the
    output sliced to partitions [32g, 32g+32) (g = p//32, tile_position
    col-group g) the matvec lands on PSUM partition p while accumulating +0
    onto the other rows of its group. Rows are emitted round-robin over the
    4 column groups so consecutive matmuls execute concurrently in disjoint
    32-column strips of the PE array. All 256 matmuls form one accumulation
    group into a single PSUM bank => scores end up dense [i(part), j(free)]
    with rows (p, p+128) packed as the two 256-wide halves of the free axis.
  * softmax: fused (scores+b2)*mask via scalar_tensor_tensor straight
    from PSUM, ACT Exp (fp16 out) with accum_out giving row sums for
    free, reciprocal + per-partition scale; processed per i-half so the
    two halves pipeline across engines.
  * ctx^T = x(chunks as lhsT) @ w^T (w transposed 128x128 via PE),
    proj = ctx^T(lhsT) @ Wp -> [i(part), e(free)], + bp, LayerNorm via
    bn_stats/bn_aggr, rstd = Exp(-0.5*Ln(var+eps)), residual add.

R tiles, the hi/hj matmul operands, and the softmax/ctx/proj weights are
fp16 (PE runs fp32 matmuls at 1/4 speed; fp16 streams at 1 col/cycle and
DVE tensor_scalar gets the 16-bit perf modes). Scores accumulate in fp32
PSUM; LayerNorm statistics and the residual stay fp32.
"""

import numpy as np
from contextlib import ExitStack

import concourse.bass as bass
import concourse.bacc as bacc
import concourse.tile as tile
from concourse import mybir
from concourse.bass_utils import run_bass_kernel_spmd

B, N, D = 8, 256, 256
P = 128
LN_EPS = 1e-5
F32 = mybir.dt.float32
F16 = mybir.dt.float16
AF = mybir.ActivationFunctionType
OP = mybir.AluOpType

# Per-row engine assignment for the pairwise relu tiles, balancing the
# three elementwise-capable engines (HW rates per [128,256] op: DVE
# ~127ns at the 16-bit perf mode, ACT ~400ns, Pool ~500ns): out of every
# 32 rows, 21 go to DVE, 7 to ACT, 4 to Pool.
def _row_engine(nc, p):
    q, k = p % 32, p // 32
    if q >= 2 and k == q % 4 and (q % 8) != 7:
        return nc.scalar
    if q >= 8 and q % 2 == 0 and k == (q + 2) % 4:
        return nc.gpsimd
    return nc.vector


def _single_act_table(arch):
    """All activation funcs this kernel uses (Relu/Identity/Copy/Exp/Ln)
    live in set 6 (natural_log_exp_and_others). The stock greedy table
    placer picks sets 0/5 and ping-pongs 5 table loads (~2.7us each on
    HW); masking every other set forces one load of set 6. Canonical set
    indices are preserved (walrus maps id -> act.json by position)."""
    import concourse.hw_specs as hw_specs

    tabs = hw_specs.get_activation_tables(arch)
    keep = "natural_log_exp_and_others"
    need = {AF.Relu, AF.Identity, AF.Copy, AF.Exp, AF.Ln}
    if keep not in tabs or not need.issubset(tabs[keep]):
        return tabs  # fall back to the stock placement
    return {name: (funcs if name == keep else set()) for name, funcs in tabs.items()}


def _build_program(b2_val: float, use_gamma: bool, use_beta: bool):
    nc = bacc.Bacc("TRN2", target_bir_lowering=False, debug=False)

    xb_d = nc.dram_tensor("xb", [N, D], F32, kind="ExternalInput")
    xt16_d = nc.dram_tensor("xt16", [D, N], F16, kind="ExternalInput")
    x16_d = nc.dram_tensor("x16", [N, D], F16, kind="ExternalInput")
    w1a_d = nc.dram_tensor("w1a", [D, D], F16, kind="ExternalInput")
    w1b_d = nc.dram_tensor("w1b", [D, D], F16, kind="ExternalInput")
    ab1_d = nc.dram_tensor("ab1c", [P, 2], F32, kind="ExternalInput")
    sb0_d = nc.dram_tensor("sb0", [P, 64], F16, kind="ExternalInput")
    sb1_d = nc.dram_tensor("sb1", [P, 64], F16, kind="ExternalInput")
    mask_d = nc.dram_tensor("maskd", [P, N * 2], F32, kind="ExternalInput")
    ident16_d = nc.dram_tensor("ident16", [P, P], F16, kind="ExternalInput")
    wp_d = nc.dram_tensor("wp", [D, D], F16, kind="ExternalInput")
    bpr_d = nc.dram_tensor("bpr", [P, D], F32, kind="ExternalInput")
    xpb_d = (
        nc.dram_tensor("xpb", [N, D], F32, kind="ExternalInput")
        if use_beta
        else None
    )
    gam_d = (
        nc.dram_tensor("gamr", [P, D], F32, kind="ExternalInput")
        if use_gamma
        else None
    )
    out_d = nc.dram_tensor("out", [N, D], F32, kind="ExternalOutput")

    with tile.TileContext(nc) as tc, ExitStack() as ctx:
        const = ctx.enter_context(tc.tile_pool(name="const", bufs=1))
        rpool = ctx.enter_context(tc.tile_pool(name="rtiles", bufs=16))
        rslow = ctx.enter_context(tc.tile_pool(name="rslow", bufs=16))
        ppre = ctx.enter_context(tc.tile_pool(name="ppre", bufs=2, space="PSUM"))
        pscore = ctx.enter_context(
            tc.tile_pool(name="pscore", bufs=1, space="PSUM")
        )

        # per-partition scalar constants for activation bias operands
        zero1 = const.tile([P, 1], F32)
        nc.vector.memset(zero1, 0.0)
        eps1 = const.tile([P, 1], F32)
        nc.vector.memset(eps1, LN_EPS)
        b2v = const.tile([P, 1], F32)
        nc.vector.memset(b2v, b2_val)
        # dummy activation with no data deps: forces the one ACT table load
        # (natural_log_exp set, ~1.3us) to run at t~0 instead of gating the
        # first real ACT op
        warm = const.tile([P, 1], F32)
        nc.scalar.activation(warm, zero1, AF.Relu, bias=zero1[:, 0:1])

        # ------- input DMAs needed before the main loop -------------------
        # x^T and the fp16 copy of x are host-side layout transforms of the
        # per-core shard (no FLOPs) - DMA them directly.
        # chunk pairs merged into single DMAs (DMA init latency dominates
        # these small transfers): tile [128, 2, 256], block c = rows
        # [128c, 128c+128) of the dram tensor
        xT_all = const.tile([P, 2, N], F16)
        nc.sync.dma_start(xT_all, xt16_d[:].rearrange("(c p) n -> p c n", p=P))
        xT = [xT_all[:, c, :] for c in range(2)]
        w1a_all = const.tile([P, 2, D], F16)
        nc.scalar.dma_start(w1a_all, w1a_d[:].rearrange("(c p) n -> p c n", p=P))
        w1a = [w1a_all[:, c, :] for c in range(2)]
        w1b_all = const.tile([P, 2, D], F16)
        nc.gpsimd.dma_start(w1b_all, w1b_d[:].rearrange("(c p) n -> p c n", p=P))
        w1b = [w1b_all[:, c, :] for c in range(2)]
        sb = [const.tile([P, 64], F16, tag=f"sb{c}", name=f"sb{c}") for c in range(2)]
        nc.sync.dma_start(sb[0], sb0_d[:])
        nc.sync.dma_start(sb[1], sb1_d[:])
        ab1c = const.tile([P, 2], F32)
        nc.sync.dma_start(ab1c, ab1_d[:])
        # ---------------- hiW (fp32 scalars), hjbW (fp16 stream) ----------
        # hiW[e,i] = sum_d (W1a*a)[d,e] x[i,d] ; hjbW[e,j] = ... + a*b1
        hiW = [const.tile([P, N], F32, tag=f"hiW{c}", name=f"hiW{c}") for c in range(2)]
        hjbW = [const.tile([P, N], F16, tag=f"hjbW{c}", name=f"hjbW{c}") for c in range(2)]
        for ec in range(2):
            ph = ppre.tile([P, N], F32, tag="mm")
            for dc in range(2):
                nc.tensor.matmul(
                    ph,
                    w1a[dc][:, ec * P : (ec + 1) * P],
                    xT[dc],
                    start=(dc == 0),
                    stop=(dc == 1),
                )
            nc.vector.tensor_copy(hiW[ec], ph)
        for ec in range(2):
            ph = ppre.tile([P, N], F32, tag="mm")
            for dc in range(2):
                nc.tensor.matmul(
                    ph,
                    w1b[dc][:, ec * P : (ec + 1) * P],
                    xT[dc],
                    start=(dc == 0),
                    stop=(dc == 1),
                )
            # + a*b1 (per-partition bias) while converting to fp16
            nc.scalar.activation(
                hjbW[ec], ph, AF.Identity, bias=ab1c[:, ec : ec + 1]
            )

        # ---------------- pairwise scores --------------------------------
        # psum_s[p, h*256+j] = scores[i=p+128h, j]
        # Rb layout: [c0h0 | c0h1 | c1h0 | c1h1], each 256 wide.
        def emit_row(pool, p):
            rb = pool.tile([P, 4 * N], F16, tag="Rb", name=f"Rb{p}")
            eng = _row_engine(nc, p)
            for c in range(2):
                for h in range(2):
                    i = p + P * h
                    dst = rb[:, (c * 2 + h) * N : (c * 2 + h + 1) * N]
                    if eng is nc.scalar:
                        nc.scalar.activation(
                            dst, hjbW[c], AF.Relu, bias=hiW[c][:, i : i + 1]
                        )
                    else:
                        eng.tensor_scalar(
                            out=dst,
                            in0=hjbW[c],
                            scalar1=hiW[c][:, i : i + 1],
                            scalar2=0.0,
                            op0=OP.add,
                            op1=OP.max,
                        )
            return rb

        # Pool rows are ~4x slower per op than DVE; emit them all up front
        # (their own 16-slot pool) so the Q7 cores grind through them in the
        # background while the PE accumulation chain advances quad by quad.
        pre_rbs = {}
        for p in range(P):
            if _row_engine(nc, p) is nc.gpsimd:
                pre_rbs[p] = emit_row(rslow, p)

        psum_s = pscore.tile([P, 2 * N], F32)
        for q in range(32):
            rows = [q, q + 32, q + 64, q + 96]
            rbs = []
            for k, p in enumerate(rows):
                rbs.append(pre_rbs.pop(p, None) or emit_row(rpool, p))
            for c in range(2):
                for k, p in enumerate(rows):
                    m = p & 31
                    nc.tensor.matmul(
                        psum_s[32 * k : 32 * (k + 1), :],
                        sb[c][:, 32 - m : 64 - m],
                        rbs[k][:, c * 2 * N : (c * 2 + 2) * N],
                        start=(q == 0 and c == 0),
                        stop=(q == 31 and c == 1),
                        tile_position=(0, 32 * k),
                        skip_group_check=True,
                    )

        # DMAs needed only from the softmax stage on. Emitted AFTER the main
        # loop: HWDGE waits are per-queue thresholds, so anything emitted
        # before the hiW/hjbW matmuls on these queues would gate them.
        x = [const.tile([P, D], F32, tag=f"x{c}", name=f"x{c}") for c in range(2)]
        nc.sync.dma_start(x[0], xb_d[0:P, :])
        nc.sync.dma_start(x[1], xb_d[P : 2 * P, :])
        x16_all = const.tile([P, 2, D], F16)
        nc.sync.dma_start(x16_all, x16_d[:].rearrange("(c p) n -> p c n", p=P))
        x16 = [x16_all[:, c, :] for c in range(2)]
        ident16 = const.tile([P, P], F16)
        nc.sync.dma_start(ident16, ident16_d[:])
        maskd = const.tile([P, 2 * N], F32)
        nc.sync.dma_start(maskd, mask_d[:])
        wp16_all = const.tile([P, 2, D], F16)
        nc.sync.dma_start(wp16_all, wp_d[:].rearrange("(c p) n -> p c n", p=P))
        wp16 = [wp16_all[:, c, :] for c in range(2)]
        bpr = const.tile([P, D], F32)
        nc.sync.dma_start(bpr, bpr_d[:])
        if use_beta:
            xpb = [
                const.tile([P, D], F32, tag=f"xpb{c}", name=f"xpb{c}")
                for c in range(2)
            ]
            for c in range(2):
                nc.sync.dma_start(xpb[c], xpb_d[c * P : (c + 1) * P, :])
        else:
            xpb = x
        if use_gamma:
            gam = const.tile([P, D], F32)
            nc.sync.dma_start(gam, gam_d[:])

        # -------- softmax / ctx / proj / LN, pipelined per i-half ---------
        # half hh covers i in [hh*128, hh*128+128) = free columns
        # [hh*256, hh*256+256) of psum_s.
        sm2 = const.tile([P, 2 * N], F32)
        ew = const.tile([P, 2 * N], F16)
        S = const.tile([P, 2], F32)
        recip = const.tile([P, 2], F32)
        wgt = const.tile([P, 2 * N], F16)
        wT = [const.tile([P, N], F16, tag=f"wT{c}", name=f"wT{c}") for c in range(2)]
        ctxT = [const.tile([P, N], F16, tag=f"ctxT{c}", name=f"ctxT{c}") for c in range(2)]
        for hh in range(2):
            hs = slice(hh * N, (hh + 1) * N)
            # sm2 = (scores + b2) * diag_mask, fused, straight from PSUM
            nc.vector.scalar_tensor_tensor(
                out=sm2[:, hs], in0=psum_s[:, hs], scalar=b2_val,
                in1=maskd[:, hs], op0=OP.add, op1=OP.mult,
            )
            nc.scalar.activation(
                ew[:, hs], sm2[:, hs], AF.Exp,
                bias=zero1[:, 0:1], accum_out=S[:, hh : hh + 1],
            )
            nc.vector.reciprocal(recip[:, hh : hh + 1], S[:, hh : hh + 1])
            nc.vector.tensor_scalar(
                out=wgt[:, hs], in0=ew[:, hs],
                scalar1=recip[:, hh : hh + 1], scalar2=None, op0=OP.mult,
            )
            # transpose this half's 128x256 block of w
            for cj in range(2):
                tp16 = ppre.tile([P, P], F16, tag="tp")
                nc.tensor.transpose(
                    tp16, wgt[:, hh * N + cj * P : hh * N + (cj + 1) * P], ident16
                )
                nc.scalar.copy(wT[cj][:, hh * P : (hh + 1) * P], tp16)
            # ctx^T[d, i-half] = sum_j x[j,d] w[i,j]
            for dc in range(2):
                pc = ppre.tile([P, P], F32, tag="mm")
                for jc in range(2):
                    nc.tensor.matmul(
                        pc,
                        x16[jc][:, dc * P : (dc + 1) * P],
                        wT[jc][:, hh * P : (hh + 1) * P],
                        start=(jc == 0),
                        stop=(jc == 1),
                    )
                nc.scalar.copy(ctxT[dc][:, hh * P : (hh + 1) * P], pc)
            # proj[i-half, e] then LayerNorm + residual
            pp = ppre.tile([P, N], F32, tag="mm")
            for dc in range(2):
                nc.tensor.matmul(
                    pp,
                    ctxT[dc][:, hh * P : (hh + 1) * P],
                    wp16[dc],
                    start=(dc == 0),
                    stop=(dc == 1),
                )
            pb = const.tile([P, D], F32, tag=f"pb{hh}", name=f"pb{hh}")
            nc.vector.tensor_tensor(out=pb, in0=pp, in1=bpr, op=OP.add)
            st = const.tile([P, 6], F32, tag=f"st{hh}", name=f"st{hh}")
            nc.vector.bn_stats(st, pb)
            mv = const.tile([P, 2], F32, tag=f"mv{hh}", name=f"mv{hh}")
            nc.vector.bn_aggr(mv, st)
            lnv = const.tile([P, 1], F32, tag=f"lnv{hh}", name=f"lnv{hh}")
            nc.scalar.activation(lnv, mv[:, 1:2], AF.Ln, bias=eps1[:, 0:1])
            rstd = const.tile([P, 1], F32, tag=f"rstd{hh}", name=f"rstd{hh}")
            nc.scalar.activation(rstd, lnv, AF.Exp, bias=zero1[:, 0:1], scale=-0.5)
            nmr = const.tile([P, 1], F32, tag=f"nmr{hh}", name=f"nmr{hh}")
            nc.vector.tensor_scalar(
                out=nmr,
                in0=mv[:, 0:1],
                scalar1=rstd[:, 0:1],
                scalar2=-1.0,
                op0=OP.mult,
                op1=OP.mult,
            )
            tt = const.tile([P, D], F32, tag=f"tt{hh}", name=f"tt{hh}")
            nc.vector.tensor_scalar(
                out=tt,
                in0=pb,
                scalar1=rstd[:, 0:1],
                scalar2=nmr[:, 0:1],
                op0=OP.mult,
                op1=OP.add,
            )
            if use_gamma:
                tg = const.tile([P, D], F32, tag=f"tg{hh}", name=f"tg{hh}")
                nc.vector.tensor_tensor(out=tg, in0=tt, in1=gam, op=OP.mult)
                tt = tg
            ot = const.tile([P, D], F32, tag=f"ot{hh}", name=f"ot{hh}")
            nc.vector.tensor_tensor(out=ot, in0=tt, in1=xpb[hh], op=OP.add)
            nc.sync.dma_start(out_d[hh * P : (hh + 1) * P, :], ot)

    import concourse.bacc as _bacc_mod

    orig = _bacc_mod.get_activation_tables
    _bacc_mod.get_activation_tables = _single_act_table
    try:
        nc.compile()
    finally:
        _bacc_mod.get_activation_tables = orig
    return nc


_cache = {}


def _get_program(b2_val: float, use_gamma: bool, use_beta: bool):
    key = (b2_val, use_gamma, use_beta)
    if key not in _cache:
        _cache[key] = _build_program(b2_val, use_gamma, use_beta)
    return _cache[key]


def _host_inputs(inputs):
    x = np.ascontiguousarray(np.asarray(inputs["patch_features"], np.float32))
    W1 = np.asarray(inputs["W1"], np.float32)
    b1 = np.asarray(inputs["b1"], np.float32)
    W2 = np.asarray(inputs["W2"], np.float32).reshape(-1)
    b2 = float(np.asarray(inputs["b2"], np.float32).reshape(-1)[0])
    Wp = np.ascontiguousarray(np.asarray(inputs["Wp"], np.float32))
    bp = np.asarray(inputs["bp"], np.float32)
    gam = np.asarray(inputs["ln_gamma"], np.float32)
    bet = np.asarray(inputs["ln_beta"], np.float32)

    a = np.abs(W2)
    sig = np.where(W2 >= 0.0, 1.0, -1.0).astype(np.float32)
    w1a = np.ascontiguousarray((W1[:D] * a[None, :]).astype(np.float16))
    w1b = np.ascontiguousarray((W1[D:] * a[None, :]).astype(np.float16))
    ab1c = np.ascontiguousarray((a * b1).reshape(2, P).T)  # [P, 2]
    sbs = []
    for c in range(2):
        m = np.zeros((P, 64), np.float16)
        m[:, 32] = sig[c * P : (c + 1) * P].astype(np.float16)
        sbs.append(m)
    mask = np.ones((P, 2 * N), np.float32)
    for p in range(P):
        mask[p, p] = 0.0
        mask[p, N + P + p] = 0.0
    ident16 = np.eye(P, dtype=np.float16)
    bpr = np.ascontiguousarray(np.broadcast_to(bp[None, :], (P, D)))
    use_gamma = not np.all(gam == 1.0)
    use_beta = not np.all(bet == 0.0)
    gamr = np.ascontiguousarray(np.broadcast_to(gam[None, :], (P, D)))

    common = {
        "w1a": w1a,
        "w1b": w1b,
        "ab1c": ab1c,
        "sb0": sbs[0],
        "sb1": sbs[1],
        "maskd": mask,
        "ident16": ident16,
        "wp": Wp.astype(np.float16),
        "bpr": bpr,
    }
    if use_gamma:
        common["gamr"] = gamr
    in_maps = []
    for b in range(B):
        m = dict(common)
        m["xb"] = np.ascontiguousarray(x[b])
        m["xt16"] = np.ascontiguousarray(x[b].T.astype(np.float16))
        m["x16"] = np.ascontiguousarray(x[b].astype(np.float16))
        if use_beta:
            m["xpb"] = np.ascontiguousarray(x[b] + bet[None, :])
        in_maps.append(m)
    return in_maps, b2, use_gamma, use_beta


def _run(inputs, trace=False, tmpdir=None):
    in_maps, b2, use_gamma, use_beta = _host_inputs(inputs)
    nc = _get_program(b2, use_gamma, use_beta)
    res = run_bass_kernel_spmd(
        nc, in_maps, list(range(B)), trace=trace, tmpdir=tmpdir
    )
    out = np.stack([res.results[b]["out"] for b in range(B)]).astype(np.float32)
    return out, res


def kernel(**inputs) -> np.ndarray:
    out, _ = _run(inputs)
    return out


def predicted_time_ns():
    """Cost-model timeline estimate of one core's NEFF execution (ns)."""
    from concourse.timeline_sim import TimelineSim

    assert _cache, "run the kernel first"
    nc = next(iter(_cache.values()))
    tl = TimelineSim(nc, trace=False)
    return int(tl.simulate())
